# revision 27
# baseline (speedup 1.0000x reference)
"""Trainium2 Bass kernel for PolyNetFP4Sim.

MLP 1->64->64->32->1 with FP4-simulated weights, silu activations.
x: (1048576, 1) f32. Data-parallel over 8 cores (131072 elems each).

Per-core layout: x shard viewed as [128 rows, 1024 cols]; superchunk s
processes rows 2s (stream A) and 2s+1 (stream B) feature-major on chip.
fp32r matmuls require dst partition base 0, so streams are fused into
ONE matmul per layer via block-diagonal lhsT. L3/L4 are processed per
superchunk PAIR: DVE copies the two 64-partition L3 pre-activations
into one 128-partition SBUF tile, a single ACT does the pair's silu,
and one M=4 matmul does the pair's L4 (halving PE work there).
"""

import numpy as np

B_TOTAL = 1048576
N_CORES = 8
PER_CORE = B_TOTAL // N_CORES  # 131072
ROWS = 128
COLS = PER_CORE // ROWS  # 1024
NSUP = ROWS // 2  # 64 superchunks per core
NPAIR = NSUP // 2  # 32 pairs
HALF = COLS // 2  # 512 (one PSUM bank of f32)


def _quantize_fp4(w):
    w = np.asarray(w, dtype=np.float32)
    m, e = np.frexp(np.abs(w))
    qe = np.clip(e + 1, 0, 3)
    qm = (m >= 0.75).astype(np.float32)
    val = np.ldexp((1.0 + 0.5 * qm) * 0.5, qe - 1).astype(np.float32)
    out = np.where(w == 0, np.float32(0), np.sign(w, dtype=np.float32) * val)
    return out.astype(np.float32)


def build_program(reps=1):
    """Build the per-core Bass program (identical on all 8 cores)."""
    import concourse.bass as bass
    import concourse.bacc as bacc
    import concourse.tile as tile
    from concourse import mybir

    F32 = mybir.dt.float32
    F32R = mybir.dt.float32r
    SILU = mybir.ActivationFunctionType.Silu

    nc = bacc.Bacc("TRN2", target_bir_lowering=False, debug=False)

    x_d = nc.declare_dram_parameter("x", [ROWS, COLS], F32R, isOutput=False)
    w1_d = nc.declare_dram_parameter("w1p", [2, 128], F32R, isOutput=False)
    w2_d = nc.declare_dram_parameter("w2p", [128, 128], F32R, isOutput=False)
    w3_d = nc.declare_dram_parameter("w3p", [128, 64], F32R, isOutput=False)
    w4_d = nc.declare_dram_parameter("w4p", [128, 4], F32R, isOutput=False)
    b1_d = nc.declare_dram_parameter("b1p", [128, 1], F32, isOutput=False)
    b2_d = nc.declare_dram_parameter("b2p", [128, 1], F32, isOutput=False)
    b3_d = nc.declare_dram_parameter("b3p", [128, 1], F32, isOutput=False)
    b4_d = nc.declare_dram_parameter("b4p", [128, 1], F32, isOutput=False)
    y_d = nc.declare_dram_parameter("y", [ROWS, COLS], F32, isOutput=True)

    with tile.TileContext(nc) as tc:
        with (
            tc.tile_pool(name="wpool", bufs=1) as wp,
            tc.tile_pool(name="apool", bufs=2) as apl,
            tc.tile_pool(name="ppool", bufs=1, space=bass.MemorySpace.PSUM) as pp,
        ):
            w1t = wp.tile([2, 128], F32R, tag="w1")
            w2t = wp.tile([128, 128], F32R, tag="w2")
            w3t = wp.tile([128, 64], F32R, tag="w3")
            w4t = wp.tile([128, 4], F32R, tag="w4")
            b1t = wp.tile([128, 1], F32, tag="b1")
            b2t = wp.tile([128, 1], F32, tag="b2")
            b3t = wp.tile([128, 1], F32, tag="b3")
            b4t = wp.tile([128, 1], F32, tag="b4")
            for t, d in (
                (w1t, w1_d), (w2t, w2_d), (w3t, w3_d), (w4t, w4_d),
                (b1t, b1_d), (b2t, b2_d), (b3t, b3_d), (b4t, b4_d),
            ):
                nc.gpsimd.dma_start(t[:, :], d[:, :])

            halves = (slice(0, HALF), slice(HALF, COLS))

            # Deep software pipeline (stage offsets relative to iter `it`):
            #   DMA xt(it+2); PE: M3(it-1), M2(it), M1(it+1), M4(q) on even
            #   its; ACT: A3(q) on odd its, A2(it), A1(it+1); DVE:
            #   dveCp(it-1), dveAdd(q). Every cross-engine dep has >=1
            #   iteration of slack except the tail of each iteration's own
            #   M1->A1 / M2->A2 chain, which lands in engine-idle windows.
            for rep in range(reps):
                xt_t, h1p_t, h1s_t, h2s_t, h3yp_t = {}, {}, {}, {}, {}
                stg_t, h3s_t = {}, {}
                for it in range(-2, NSUP + 3):
                    s_dma = it + 2
                    if 0 <= s_dma < NSUP:
                        xt = apl.tile([2, COLS], F32R, tag="xt")
                        nc.gpsimd.dma_start(
                            xt[0:2, :], x_d[2 * s_dma : 2 * s_dma + 2, :]
                        )
                        xt_t[s_dma] = xt
                    s_m3 = it - 1
                    if 0 <= s_m3 < NSUP:
                        h3yp = pp.tile([128, COLS], F32, tag="h3yp")
                        for sl in halves:
                            nc.tensor.matmul(
                                h3yp[0:64, sl], w3t[0:128, 0:64],
                                h2s_t[s_m3][0:128, sl],
                                start=True, stop=True, tile_position=(0, 0),
                            )
                        h3yp_t[s_m3] = h3yp
                        del h2s_t[s_m3]
                        p, j = divmod(s_m3, 2)
                        if j == 0:
                            stg = apl.tile([128, COLS], F32, tag="stg")
                            stg_t[p] = stg
                        nc.vector.tensor_copy(
                            stg_t[p][64 * j : 64 * j + 64, :], h3yp[0:64, :]
                        )
                        del h3yp_t[s_m3]
                    if it % 2 == 1:
                        q = (it - 3) // 2
                        if 0 <= q < NPAIR:
                            h3s = apl.tile([128, COLS], F32R, tag="h3s")
                            nc.scalar.activation(
                                h3s[:, :], stg_t[q][:, :], SILU, bias=b3t[:, 0:1]
                            )
                            h3s_t[q] = h3s
                            del stg_t[q]
                    s_m2 = it
                    if 0 <= s_m2 < NSUP:
                        h2p = pp.tile([128, COLS], F32, tag="h2p")
                        for sl in halves:
                            nc.tensor.matmul(
                                h2p[0:128, sl], w2t[0:128, 0:128],
                                h1s_t[s_m2][0:128, sl],
                                start=True, stop=True, tile_position=(0, 0),
                            )
                        del h1s_t[s_m2]
                        h2s = apl.tile([128, COLS], F32R, tag="h2s")
                        nc.scalar.activation(
                            h2s[:, :], h2p[:, :], SILU, bias=b2t[:, 0:1]
                        )
                        h2s_t[s_m2] = h2s
                    s_m1 = it + 1
                    if 0 <= s_m1 < NSUP:
                        h1p = pp.tile([128, COLS], F32, tag="h1p")
                        for sl in halves:
                            nc.tensor.matmul(
                                h1p[0:128, sl], w1t[0:2, 0:128],
                                xt_t[s_m1][0:2, sl],
                                start=True, stop=True, tile_position=(0, 0),
                            )
                        del xt_t[s_m1]
                        h1s = apl.tile([128, COLS], F32R, tag="h1s")
                        nc.scalar.activation(
                            h1s[:, :], h1p[:, :], SILU, bias=b1t[:, 0:1]
                        )
                        h1s_t[s_m1] = h1s
                    if it % 2 == 0:
                        q = (it - 4) // 2
                        if 0 <= q < NPAIR:
                            yp = pp.tile([128, COLS], F32, tag="yp")
                            for sl in halves:
                                nc.tensor.matmul(
                                    yp[0:4, sl], w4t[0:128, 0:4],
                                    h3s_t[q][0:128, sl],
                                    start=True, stop=True, tile_position=(0, 0),
                                )
                            del h3s_t[q]
                            yo = apl.tile([4, COLS], F32, tag="yo")
                            nc.vector.tensor_scalar_add(
                                yo[0:4, :], yp[0:4, :], b4t[0:4, 0:1]
                            )
                            nc.gpsimd.dma_start(
                                y_d[4 * q : 4 * q + 4, :], yo[0:4, :]
                            )
    nc.compile()
    return nc


def pack_inputs(x, w1, b1, w2, b2, w3, b3, w4, b4):
    """Quantize weights and pack into the on-chip partition layouts."""
    qw1 = _quantize_fp4(w1)  # (64, 1)
    qw2 = _quantize_fp4(w2)  # (64, 64)
    qw3 = _quantize_fp4(w3)  # (32, 64)
    qw4 = _quantize_fp4(w4)  # (1, 32)

    w1p = np.zeros((2, 128), np.float32)
    w1p[0, 0:64] = qw1[:, 0]
    w1p[1, 64:128] = qw1[:, 0]
    w2p = np.zeros((128, 128), np.float32)
    w2p[0:64, 0:64] = qw2.T
    w2p[64:128, 64:128] = qw2.T
    w3p = np.zeros((128, 64), np.float32)
    w3p[0:64, 0:32] = qw3.T
    w3p[64:128, 32:64] = qw3.T
    w4p = np.zeros((128, 4), np.float32)
    for j in range(4):
        w4p[32 * j : 32 * j + 32, j] = qw4[0, :]

    b1p = np.zeros((128, 1), np.float32)
    b1p[0:64, 0] = b1
    b1p[64:128, 0] = b1
    b2p = np.zeros((128, 1), np.float32)
    b2p[0:64, 0] = b2
    b2p[64:128, 0] = b2
    b3p = np.zeros((128, 1), np.float32)
    for j in range(4):
        b3p[32 * j : 32 * j + 32, 0] = b3
    b4p = np.full((128, 1), np.float32(b4[0]), np.float32)

    x_flat = np.ascontiguousarray(x, dtype=np.float32).reshape(-1)
    shards = [
        x_flat[i * PER_CORE : (i + 1) * PER_CORE].reshape(ROWS, COLS)
        for i in range(N_CORES)
    ]
    weights = {
        "w1p": w1p, "w2p": w2p, "w3p": w3p, "w4p": w4p,
        "b1p": b1p, "b2p": b2p, "b3p": b3p, "b4p": b4p,
    }
    in_maps = [{"x": shards[i], **weights} for i in range(N_CORES)]
    return in_maps


def run(inputs, trace=False, trace_kwargs=None, tmpdir=None, reps=1):
    from concourse.bass_utils import run_bass_kernel_spmd

    in_maps = pack_inputs(**inputs)
    nc = build_program(reps=reps)
    kw = {}
    if trace:
        kw["trace"] = True
        if trace_kwargs:
            kw["trace_kwargs"] = trace_kwargs
        if tmpdir:
            kw["tmpdir"] = tmpdir
    res = run_bass_kernel_spmd(nc, in_maps, list(range(N_CORES)), **kw)
    y = np.concatenate([res.results[i]["y"].reshape(-1) for i in range(N_CORES)])
    return y.reshape(B_TOTAL, 1).astype(np.float32), res


def kernel(**inputs):
    y, _ = run(inputs, trace=False)
    return y
